# revision 33
# baseline (speedup 1.0000x reference)
"""SnakeHead Trainium2 kernel.

Model (per batch): bilinear-sample a [256,256,126] feature map at 1024
vertices, concat the (y,x) coords -> [1024,128], 1x1 conv to 512 + ReLU,
six dilated (1,3,9,9,3,1) kernel-3 conv1d layers 512->512 + ReLU, final
1x1 conv 512->2.

Strategy: data-parallel over batch, 2 batches per NeuronCore (16/8).
On each core:
  - host pre-expands the feature map into 2x2-neighborhood rows
    (fm_g[y,x] = fm[y:y+2, x:x+2] flattened, 504 floats): the whole
    bilinear stencil for one vertex is ONE 2016-byte indirect-DMA
    descriptor; 8 gather calls per batch ([P,1] offsets)
  - bilinear-combine on the vector engine, vertices-on-partitions
  - PE-transpose to channel-major [ch, tok] layout
  - all convs as float32r matmuls (full PE rate), channels on
    partitions, tokens on the free axis; dilated taps are just shifted
    rhs slices into zero-padded halo activation buffers; each weight
    block stays loaded across both 512-token segments
  - bias+ReLU fused in the scalar-engine PSUM->SBUF eviction
  - layer-weight DMAs are WAR-gated behind early gather landings so the
    latency-critical gather stream owns the DMA engines first
"""

import numpy as np
from contextlib import ExitStack

import concourse.bass as bass
import concourse.bacc as bacc
import concourse.mybir as mybir
import concourse.tile as tile
from concourse.bass import IndirectOffsetOnAxis
from concourse.bass_utils import run_bass_kernel_spmd
from concourse.masks import make_identity

P = 128
B, N, H, W, Cf, Ch = 16, 1024, 256, 256, 126, 512
NCORES = 8
BPC = B // NCORES          # batches per core
T = BPC * N                # tokens per core
D = Cf + 2                 # input channels to layer 0
DILS = (1, 3, 9, 9, 3, 1)
PAD = 16                   # halo >= max dilation (9)
SEG = PAD + N + PAD        # per-batch activation columns
NT = T // P                # 128-token tiles per core
NTB = N // P               # 128-token tiles per batch (8)
CB = Ch // P               # 128-channel blocks (4)
HALF = 512                 # matmul moving-dim tile (tokens)
RG = BPC * H * W           # fm_g rows (one 504-float row per pixel)

F32 = mybir.dt.float32
FR = mybir.dt.float32r
I32 = mybir.dt.int32
AF = mybir.ActivationFunctionType
ALU = mybir.AluOpType


def build_program(reps=1, nlayers=6, wbufs=2):
    nc = bacc.Bacc(trn_type="TRN2", target_bir_lowering=False)

    verts = nc.declare_dram_parameter("verts", [P, NT * 2], F32, False)
    gidx = nc.declare_dram_parameter("gidx", [P, NT], I32, False)
    gw = nc.declare_dram_parameter("gw", [P, NT * 4], F32, False)
    fmg = nc.declare_dram_parameter("fmg", [RG, 4 * Cf], F32, False)
    w0 = nc.declare_dram_parameter("w0", [P, Ch], FR, False)
    b0 = nc.declare_dram_parameter("b0", [P, CB], F32, False)
    ws = nc.declare_dram_parameter("ws", [6, P, 3 * CB * Ch], FR, False)
    bs = nc.declare_dram_parameter("bs", [P, 6 * CB], F32, False)
    woff = nc.declare_dram_parameter("woff", [P, CB * 2], FR, False)
    out = nc.declare_dram_parameter("out", [2, T], F32, True)

    with tile.TileContext(nc) as tc, ExitStack() as ctx:
        const = ctx.enter_context(tc.tile_pool(name="const", bufs=1))
        work = ctx.enter_context(tc.tile_pool(name="work", bufs=1))
        gpool = ctx.enter_context(tc.tile_pool(name="gpool", bufs=4))
        wpool = ctx.enter_context(tc.tile_pool(name="wpool", bufs=wbufs))
        hpool = ctx.enter_context(tc.tile_pool(name="hpool", bufs=1))
        psum = ctx.enter_context(tc.tile_pool(name="psum", bufs=2, space="PSUM"))
        for _ in range(reps):
            _emit_body(nc, tc, const, work, gpool, wpool, hpool, psum,
                       verts, gidx, gw, fmg, w0, b0, ws, bs, woff, out, nlayers)

    # Clear all kernel semaphores + DMA queues at the tail so the loaded
    # NEFF can be re-executed: without this, a second nrt_execute starts
    # with end-of-run semaphore values and every wait is pre-satisfied.
    nc.reset()
    nc.finalize()
    return nc


def _emit_body(nc, tc, const, work, gpool, wpool, hpool, psum,
               verts, gidx, gw, fmg, w0, b0, ws, bs, woff, out, nlayers=6):
    if True:
        # ---- host-precomputed gather indices first: the gather is the
        # critical path and starts as soon as this 8 KB DMA lands ----
        idx_sb = const.tile([P, NT], I32)
        # scalar-engine HWDGE: the sync queue is still busy with its
        # startup drain at this point, the scalar queue is free ~2us
        # earlier -- and the whole gather chain hangs off this DMA
        nc.scalar.dma_start(out=idx_sb[:], in_=gidx[:])
        v_sb = const.tile([P, NT * 2], F32)                   # [p, (j) yx]
        nc.sync.dma_start(out=v_sb[:], in_=verts[:])
        gw_sb = const.tile([P, NT * 4], F32)                  # w00,w01,w10,w11
        nc.sync.dma_start(out=gw_sb[:], in_=gw[:])
        v3 = v_sb[:].rearrange("p (j t) -> p j t", t=2)       # [128, 16, 2]

        # layer-weight tiles allocated up-front so the second WAR gate
        # below can reference layer 2's buffer before its DMAs are emitted
        wtiles = [wpool.tile([P, 3 * CB * Ch], FR, tag="wlayer",
                             name=f"wl{li}") for li in range(nlayers)]
        wgate2 = work.tile([P, 1], F32)

        # ---- gather: one indirect DMA per 128-token tile ----
        # each descriptor reads the full 2x2 corner block (504 f32,
        # 2016 B); 128 descriptors per call.
        cg = []
        for j in range(NT):
            ct = gpool.tile([P, 4 * Cf], F32, name="cg", tag="cg", bufs=16)
            nc.gpsimd.indirect_dma_start(
                out=ct[:], out_offset=None, in_=fmg[:],
                in_offset=IndirectOffsetOnAxis(ap=idx_sb[:, j:j + 1], axis=0))
            cg.append(ct)
            if j == NTB and nlayers > 1:
                # second gate, on the (in-order) Pool queue: layer 2+
                # weight streams wait for b1's first gather landing,
                # keeping the bandwidth-critical [10,28]us window to
                # b0's gather + layer-1 weights only.
                nc.gpsimd.tensor_tensor(
                    out=wgate2[:], in0=wtiles[1][:, 0:1], in1=ct[:, 0:1],
                    op=ALU.add)

        w00 = gw_sb[:, 0 * NT:1 * NT]
        w01 = gw_sb[:, 1 * NT:2 * NT]
        w10 = gw_sb[:, 2 * NT:3 * NT]
        w11 = gw_sb[:, 3 * NT:4 * NT]

        # ---- small loads / constants ----
        ident = const.tile([P, P], F32)
        make_identity(nc, ident[:])
        w0_sb = const.tile([P, Ch], FR)
        nc.sync.dma_start(out=w0_sb[:], in_=w0[:])
        b0_sb = const.tile([P, CB], F32)
        nc.sync.dma_start(out=b0_sb[:], in_=b0[:])
        bs_sb = const.tile([P, 6 * CB], F32)
        nc.sync.dma_start(out=bs_sb[:], in_=bs[:])
        woff_sb = const.tile([P, CB * 2], FR)
        nc.sync.dma_start(out=woff_sb[:], in_=woff[:])

        # ---- activation halo buffers (ping/pong) ----
        h = [[[hpool.tile([P, SEG], FR, name=f"h{g}_{ci}_{b}", tag=f"h{g}_{ci}_{b}")
               for b in range(BPC)] for ci in range(CB)] for g in range(2)]
        zeros_f32 = const.tile([P, PAD], F32)

        def emit_h_pads():
            # emitted after batch 0's bilinear chain so the vector engine
            # prioritizes the PE-critical x_in path; pads only gate the
            # first dilated layer's halo reads.
            nc.vector.memset(zeros_f32[:], 0.0)
            for g in range(2):
                for ci in range(CB):
                    for b in range(BPC):
                        nc.vector.tensor_copy(h[g][ci][b][:, 0:PAD], zeros_f32[:])
                        nc.vector.tensor_copy(h[g][ci][b][:, PAD + N:SEG], zeros_f32[:])

        # ---- layer weight prefetch, WAR-gated behind the first gather ----
        # the dummy read of wt[0] (before its DMA) also reads the first
        # gather tile, so the weight streams -- via sync-engine FIFO
        # order -- queue behind the latency-critical first gather
        # descriptors instead of ahead of them. Each layer is loaded in
        # 12 chunk-DMAs of [P,512] so the SDMA engines' packet-granular
        # round-robin shares bandwidth fairly with the 2 KB gather
        # descriptors (a single [P,6144] DMA makes 24.6 KB descriptors
        # that starve the gather stream ~6:1).
        wgate = work.tile([P, 1], F32)
        wcur = []
        for li in range(nlayers):
            wt = wtiles[li]
            if li == 0:
                nc.vector.tensor_tensor(
                    out=wgate[:], in0=wt[:, 0:1], in1=cg[0][:, 0:1],
                    op=ALU.add)
            for ch in range(3 * CB):
                nc.sync.dma_start(out=wt[:, ch * Ch:(ch + 1) * Ch],
                                  in_=ws[li][:, ch * Ch:(ch + 1) * Ch])
            wcur.append(wt)

        x_in = const.tile([P, T], FR)

        def emit_bilinear(b):
            """bilinears for batch b (vector); transposes emitted separately
            so the in-order vector queue never blocks on a PE transpose."""
            xpres = []
            for jj in range(NTB):
                j = b * NTB + jj
                c = cg[j]
                xpre = gpool.tile([P, P], F32, tag="xpre", bufs=8)
                nc.vector.tensor_scalar(
                    out=xpre[:, 0:Cf], in0=c[:, 0:Cf],
                    scalar1=w00[:, j:j + 1], scalar2=None, op0=ALU.mult)
                for cq, wq in ((c[:, Cf:2 * Cf], w01),
                               (c[:, 2 * Cf:3 * Cf], w10),
                               (c[:, 3 * Cf:4 * Cf], w11)):
                    nc.vector.scalar_tensor_tensor(
                        out=xpre[:, 0:Cf], in0=cq,
                        scalar=wq[:, j:j + 1], in1=xpre[:, 0:Cf],
                        op0=ALU.mult, op1=ALU.add)
                nc.vector.tensor_copy(out=xpre[:, Cf:Cf + 2], in_=v3[:, j, :])
                xpres.append(xpre)
            return xpres

        def emit_transpose(b, xpres, jj):
            j = b * NTB + jj
            tp = psum.tile([P, P], F32, tag="tps", bufs=2)
            nc.tensor.transpose(out=tp[:], in_=xpres[jj][:], identity=ident[:])
            # vector (not scalar) eviction: the scalar engine is busy
            # with conv ACTs when batch 1's x_in lands
            nc.vector.tensor_copy(out=x_in[:, j * P:(j + 1) * P], in_=tp[:])

        def emit_xin_l0(b):
            xpres = emit_bilinear(b)
            for jj in range(NTB):
                emit_transpose(b, xpres, jj)

        def emit_l0(b):
            for co in range(CB):
                for s in range(N // HALF):
                    ps = psum.tile([P, HALF], F32, tag="mm", bufs=5)
                    nc.tensor.matmul(
                        ps[:],
                        lhsT=w0_sb[:, co * P:(co + 1) * P],
                        rhs=x_in[:, b * N + s * HALF:b * N + (s + 1) * HALF],
                        start=True, stop=True)
                    # evict on the (idle) vector engine: relu(x+b). The
                    # scalar engine's serial 688ns ACTs would gate the next
                    # layer's psum rotation at the L0->L1 transitions.
                    nc.vector.tensor_scalar(
                        out=h[0][co][b][:, PAD + s * HALF:PAD + (s + 1) * HALF],
                        in0=ps[:],
                        scalar1=b0_sb[:, co:co + 1], scalar2=0.0,
                        op0=ALU.add, op1=ALU.max)

        def emit_layer(li, dil, b, post_co=None):
            gin, gout = li % 2, (li + 1) % 2
            wt = wcur[li]
            for co in range(CB):
                for s in range(N // HALF):
                    ps = psum.tile([P, HALF], F32, tag="mm", bufs=5)
                    for ci in range(CB):
                        for k in range(3):
                            col = (k * CB + ci) * Ch + co * P
                            off = PAD + s * HALF + (k - 1) * dil
                            nc.tensor.matmul(
                                ps[:],
                                lhsT=wt[:, col:col + P],
                                rhs=h[gin][ci][b][:, off:off + HALF],
                                start=(ci == 0 and k == 0),
                                stop=(ci == CB - 1 and k == 2))
                    nc.scalar.activation(
                        h[gout][co][b][:, PAD + s * HALF:PAD + (s + 1) * HALF],
                        ps[:], AF.Relu,
                        bias=bs_sb[:, li * CB + co:li * CB + co + 1])
                if post_co is not None:
                    post_co(co)

        # ---- final 1x1 conv Ch->2 (no bias), output DMA per batch ----
        gfin = nlayers % 2
        out_sb = const.tile([2, T], F32)

        def emit_fin(b):
            # per-segment output DMA: the s=0 store overlaps the s=1 matmuls
            for s in range(N // HALF):
                ps = psum.tile([2, HALF], F32, tag="mm", bufs=5)
                for ci in range(CB):
                    nc.tensor.matmul(
                        ps[:],
                        lhsT=woff_sb[:, ci * 2:(ci + 1) * 2],
                        rhs=h[gfin][ci][b][:, PAD + s * HALF:PAD + (s + 1) * HALF],
                        start=(ci == 0), stop=(ci == CB - 1))
                lo = b * N + s * HALF
                nc.vector.tensor_copy(out=out_sb[:, lo:lo + HALF], in_=ps[:])
                nc.sync.dma_start(out=out[:, lo:lo + HALF],
                                  in_=out_sb[:, lo:lo + HALF])

        # ---- emission order: keep the PE queue stall-free ----
        # b0's x_in/L0 and its first TWO dilated layers run while b1's
        # gather is still in flight; from layer 2 on, batches alternate
        # within each layer (single weight load per layer, wpool-staged);
        # each batch's final conv + output DMA go right after its last
        # layer so only b1's final lands in the tail.
        emit_xin_l0(0)
        emit_l0(0)
        emit_h_pads()
        head = min(2, nlayers)
        emit_layer(0, DILS[0], 0)
        xp1 = emit_bilinear(1)

        def sprinkle(co):
            # b1's PE transposes ride between D3(b0)'s matmul bursts so
            # the PE (and the HAM clock-gate) never sees an idle window
            # at the b0->b1 transition.
            emit_transpose(1, xp1, 2 * co)
            emit_transpose(1, xp1, 2 * co + 1)

        emit_layer(1, DILS[1], 0, post_co=sprinkle)
        emit_l0(1)
        for li in range(head):
            emit_layer(li, DILS[li], 1)
        for li, dil in enumerate(DILS[:nlayers]):
            if li < head:
                continue
            for b in range(BPC):
                emit_layer(li, dil, b)
                if li == nlayers - 1:
                    emit_fin(b)
        if nlayers - 1 < head:
            for b in range(BPC):
                emit_fin(b)


def shard_inputs(vertices, feature_map, w0, b0, ws, bs, w_off):
    """Build the per-core input maps.

    fm_g expansion: fm_g[b, y, x] = fm[b, y:y+2, x:x+2, :] flattened to
    504 floats (edge rows/cols padded with zeros; never addressed since
    vertex coords are strictly inside [0, 255)).
    """
    vertices = np.ascontiguousarray(vertices, np.float32)
    feature_map = np.asarray(feature_map, np.float32)
    w0r = np.ascontiguousarray(np.asarray(w0, np.float32).reshape(D, Ch))
    b0r = np.ascontiguousarray(np.asarray(b0, np.float32).reshape(CB, P).T)
    # ws[l,k,ci*128+p,co] -> [l, p, (k ci co)]
    wsr = np.ascontiguousarray(
        np.asarray(ws, np.float32)
        .reshape(6, 3, CB, P, Ch).transpose(0, 3, 1, 2, 4).reshape(6, P, 3 * CB * Ch))
    bsr = np.ascontiguousarray(
        np.asarray(bs, np.float32).reshape(6, CB, P).transpose(2, 0, 1).reshape(P, 6 * CB))
    woffr = np.ascontiguousarray(
        np.asarray(w_off, np.float32).reshape(CB, P, 2).transpose(1, 0, 2).reshape(P, CB * 2))

    in_maps = []
    for c in range(NCORES):
        vb = vertices[c * BPC:(c + 1) * BPC]          # [BPC, N, 2]
        # [P, BPC*NTB, 2]: partition p holds token (b, jj*128 + p)
        vr3 = vb.reshape(BPC, NTB, P, 2).transpose(2, 0, 1, 3).reshape(P, NT, 2)
        vr = np.ascontiguousarray(vr3.reshape(P, NT * 2))
        # gather indices + bilinear weights (float32 math matches the
        # reference's map_coordinates up to fp32 rounding)
        coords = (vr3 + np.float32(1.0)) * np.float32(127.5)   # [P,NT,2]
        c0 = np.floor(coords).astype(np.float32)
        fr = coords - c0                                       # wy, wx
        y0i = c0[..., 0].astype(np.int64)
        x0i = c0[..., 1].astype(np.int64)
        bi = (np.arange(NT) // NTB)[None, :]                   # batch per tile
        gidx = (bi * (H * W) + y0i * W + x0i).astype(np.int32)  # [P, NT]
        wy, wx = fr[..., 0], fr[..., 1]
        gww = np.empty((P, 4, NT), np.float32)
        gww[:, 0] = (1 - wy) * (1 - wx)
        gww[:, 1] = (1 - wy) * wx
        gww[:, 2] = wy * (1 - wx)
        gww[:, 3] = wy * wx
        fmb = feature_map[c * BPC:(c + 1) * BPC]      # [BPC, H, W, Cf]
        fmp = np.zeros((BPC, H + 1, W + 1, Cf), np.float32)
        fmp[:, :H, :W] = fmb
        fmg = np.empty((BPC, H, W, 4, Cf), np.float32)
        fmg[:, :, :, 0] = fmp[:, :H, :W]
        fmg[:, :, :, 1] = fmp[:, :H, 1:W + 1]
        fmg[:, :, :, 2] = fmp[:, 1:H + 1, :W]
        fmg[:, :, :, 3] = fmp[:, 1:H + 1, 1:W + 1]
        in_maps.append({
            "verts": vr,
            "gidx": np.ascontiguousarray(gidx),
            "gw": np.ascontiguousarray(gww.reshape(P, 4 * NT)),
            "fmg": fmg.reshape(RG, 4 * Cf),
            "w0": w0r, "b0": b0r, "ws": wsr, "bs": bsr, "woff": woffr,
        })
    return in_maps


def unshard_output(results):
    outs = []
    for r in results:
        o = np.asarray(r["out"])                       # [2, T] = [ch, b*N+n]
        outs.append(o.reshape(2, BPC, N).transpose(1, 2, 0))   # [BPC, N, 2]
    return np.concatenate(outs, axis=0).astype(np.float32)


_NC_CACHE = {}


def _get_program():
    if "nc" not in _NC_CACHE:
        _NC_CACHE["nc"] = build_program()
    return _NC_CACHE["nc"]


def run(inputs, trace=False):
    nc = _get_program()
    in_maps = shard_inputs(**inputs)
    res = run_bass_kernel_spmd(nc, in_maps, list(range(NCORES)), trace=trace)
    return unshard_output(res.results), res


def kernel(**inputs) -> np.ndarray:
    out, _ = run(inputs, trace=False)
    return out
